# revision 3
# baseline (speedup 1.0000x reference)
"""Fused multi-head attention with dropout for Trainium2 (Bass/Tile), 8-core SPMD.

Problem: out = dropout(softmax(Q @ K^T * scale)) @ V
  Q/K/V: [64, 2048, 64] fp32, dropout_mask: [64, 2048, 2048] fp32, p = 0.5.

Sharding: 64 batch*heads split across 8 NeuronCores (8 heads/core), no
cross-device comms. Scores computed TRANSPOSED (S^T[k,q] = K @ Q^T) so the
softmax k-sum lands on the partition axis and PV needs no transpose.

v6 structure (HW-microbenched rates in parens):

 - 512-wide q-blocks: one PSUM st tile [128, 1024] holds TWO k-chunks side
   by side (chunk 2p at cols 0:512, 2p+1 at 512:1024), each produced by one
   row-tiled QK matmul: chunk 2p's K slab lives on partitions 0-63, chunk
   2p+1's on 64-127 (tile_position auto), Q duplicated to both halves. The
   two matmuls are adjacent in the PE queue and share a single WAR (the one
   exp that consumes the tile), so they execute concurrently (77.7 ns/mm vs
   398.8 sequential fp32r).
 - Act is the pacer: 256 exps of [128,1024] PSUM->SBUF bf16, ~1038ns each
   (~266us/core). QK runs 1 pair-tile ahead; st double-buffered (2x2 banks).
 - Whole softmax denominator on PE: each p0 tile folds into oden via 2
   accumulating ones-matmuls (164 ns/mm), zero DVE adds. The 0.5-valued
   ones fold the dropout 1/(1-p)=2 rescale: out = oacc / (0.5 * sum exp).
 - DVE: one mask-mult per tile (all-bf16 [128,1024] tensor_tensor, 408ns)
   + per-block recip/out-mult (~54% busy).
 - oacc/oden are [64,512] = 1 PSUM bank each, DOUBLE-buffered: block b+1
   accumulates into fresh banks while block b's out-stage drains, so there
   is no PSUM WAR at block boundaries. PSUM total: 4+2+2 = 8 banks.
 - Per-window budget (1.04us Act): PE 852ns (QK 156 + PV 368 + fold 328),
   DVE ~560ns, DMA ~720ns (two [128,512] bf16 mask quads).
"""

import numpy as np
from contextlib import ExitStack

import concourse.bass as bass
import concourse.bacc as bacc
import concourse.tile as tile
import concourse.mybir as mybir
from concourse.bass_utils import run_bass_kernel_spmd

N_CORES = 8
B, S, D = 64, 2048, 64
HPC = B // N_CORES  # heads per core
KP = 128            # k-chunk size (PSUM partition dim)
QB = 512            # q-block width
DROP_P = 0.5
N_KC = S // KP      # 16 k-chunks
N_PR = N_KC // 2    # 8 chunk pairs = 8 st tiles per block
MK_LEAD = 3         # mask DMA prefetch lead, in quad-tiles


def build_program(
    n_heads=HPC,
    seq=S,
    d=D,
    scale=1.0,
    reps=1,
):
    f32 = mybir.dt.float32
    bf16 = mybir.dt.bfloat16
    fmm = mybir.dt.float32r
    n_kc = seq // KP
    n_pr = n_kc // 2
    n_qb = seq // QB

    nc = bacc.Bacc("TRN2", target_bir_lowering=False, debug=False)
    qt_d = nc.dram_tensor("qt", [n_heads, KP, seq], fmm, kind="ExternalInput").ap()
    kt_d = nc.dram_tensor("kt", [n_heads, KP, seq // 2], fmm, kind="ExternalInput").ap()
    vp_d = nc.dram_tensor("vp", [n_heads, KP, n_kc * d], bf16, kind="ExternalInput").ap()
    mt_d = nc.dram_tensor("mt", [n_heads, seq, seq], bf16, kind="ExternalInput").ap()
    ot_d = nc.dram_tensor("ot", [n_heads, d, seq], f32, kind="ExternalOutput").ap()

    blocks = [(h, qb) for h in range(n_heads) for qb in range(n_qb)] * reps

    with tile.TileContext(nc) as tc:
        with ExitStack() as ctx:
            const = ctx.enter_context(tc.tile_pool(name="const", bufs=1))
            qkv = ctx.enter_context(tc.tile_pool(name="qkv", bufs=2))
            mpool = ctx.enter_context(tc.tile_pool(name="mask", bufs=5))
            ppool = ctx.enter_context(tc.tile_pool(name="p", bufs=5))
            dpool = ctx.enter_context(tc.tile_pool(name="pd", bufs=4))
            apool = ctx.enter_context(tc.tile_pool(name="acc", bufs=3))
            opool = ctx.enter_context(tc.tile_pool(name="o", bufs=4))
            # PSUM (8 banks): st 2x2 + oacc 2x1 + oden 2x1.
            pst = ctx.enter_context(
                tc.tile_pool(name="pst", bufs=2, space=bass.MemorySpace.PSUM)
            )
            pacc = ctx.enter_context(
                tc.tile_pool(name="pacc", bufs=2, space=bass.MemorySpace.PSUM)
            )
            pden = ctx.enter_context(
                tc.tile_pool(name="pden", bufs=2, space=bass.MemorySpace.PSUM)
            )

            ones = const.tile([KP, d], bf16)
            nc.vector.memset(ones[:], 0.5)

            head_tiles: dict = {}

            def load_head(h):
                qt_sb = qkv.tile([KP, seq], fmm, tag="qt")
                nc.sync.dma_start(qt_sb[:], qt_d[h])
                kt_sb = qkv.tile([KP, seq // 2], fmm, tag="kt")
                nc.sync.dma_start(kt_sb[:], kt_d[h])
                v_sb = qkv.tile([KP, n_kc * d], bf16, tag="v")
                nc.sync.dma_start(v_sb[:], vp_d[h])
                head_tiles[h] = (qt_sb, kt_sb, v_sb)

            mk_tiles: dict = {}
            st_tiles: dict = {}

            def dma_mk(b, pp):
                # one strided DMA covers FOUR chunks (two pair-tiles): halves
                # the DMA-issue queue load. dst[p, i*QB+q] = mask[(c0+i)*KP+p, q]
                h, qb = blocks[b]
                q0 = qb * QB
                c0 = 4 * pp
                t = mpool.tile([KP, 4 * QB], bf16, tag="mk")
                src = mt_d[h, c0 * KP : (c0 + 4) * KP, q0 : q0 + QB]
                nc.sync.dma_start(
                    t[:], src.rearrange("(four p) q -> p four q", four=4)
                )
                mk_tiles[(b, 2 * pp)] = t[:, 0 : 2 * QB]
                mk_tiles[(b, 2 * pp + 1)] = t[:, 2 * QB : 4 * QB]

            def qk(b, p):
                """One st tile = chunks (2p | 2p+1) x QB, via a row-tiled
                matmul pair (rows 0-63 / 64-127), adjacent in the PE queue."""
                h, qb = blocks[b]
                q0 = qb * QB
                qt_sb, kt_sb, _ = head_tiles[h]
                t = pst.tile([KP, 2 * QB], f32, tag="st")
                ks = slice(p * KP, (p + 1) * KP)
                nc.tensor.matmul(
                    t[:, 0:QB], kt_sb[0:64, ks], qt_sb[0:64, q0 : q0 + QB],
                    start=True, stop=True,
                )
                nc.tensor.matmul(
                    t[:, QB : 2 * QB], kt_sb[64:128, ks], qt_sb[64:128, q0 : q0 + QB],
                    start=True, stop=True,
                )
                st_tiles[(b, p)] = t

            mk_sched = [(bb, pp) for bb in range(len(blocks)) for pp in range(n_pr // 2)]
            mk_cursor = [0]

            def advance_mk(n):
                for _ in range(n):
                    if mk_cursor[0] < len(mk_sched):
                        dma_mk(*mk_sched[mk_cursor[0]])
                        mk_cursor[0] += 1

            load_head(0)
            advance_mk(MK_LEAD)
            qk(0, 0)

            pending = [None, None]  # deferred out-stage compute / dma

            for b, (h, qb) in enumerate(blocks):
                _, _, v_sb = head_tiles[h]
                oacc = pacc.tile([d, QB], f32, tag="oacc")
                oden = pden.tile([d, QB], f32, tag="oden")
                accs = [None, None]
                seeds = [None, None]

                for p in range(n_pr):
                    # prefetch next head's tensors ~1.5 blocks early
                    if (
                        p == n_pr // 2
                        and qb == n_qb - 2
                        and b + 2 < len(blocks)
                        and blocks[b + 2][0] != h
                    ):
                        load_head(blocks[b + 2][0])
                    if p % 2 == 0:
                        advance_mk(1)

                    st = st_tiles.pop((b, p))
                    p0 = ppool.tile([KP, 2 * QB], bf16, tag="p0")
                    nc.scalar.activation(
                        p0[:], st[:], mybir.ActivationFunctionType.Exp, scale=scale
                    )
                    nxt = (b, p + 1) if p + 1 < n_pr else (b + 1, 0)
                    if nxt[0] < len(blocks):
                        qk(*nxt)
                    mk = mk_tiles.pop((b, p))
                    pd = dpool.tile([KP, 2 * QB], bf16, tag="pd")
                    nc.vector.tensor_tensor(pd[:], mk[:], p0[:], mybir.AluOpType.mult)
                    if p == 0 and pending[0] is not None:
                        pending[1] = pending[0]()
                        pending[0] = None
                    elif p == 1 and pending[1] is not None:
                        pending[1]()
                        pending[1] = None
                    # PV: chunk 2p from pd[:, 0:QB], chunk 2p+1 from the rest
                    for half in range(2):
                        c = 2 * p + half
                        nc.tensor.matmul(
                            oacc[:],
                            v_sb[:, c * d : (c + 1) * d],
                            pd[:, half * QB : (half + 1) * QB],
                            start=c == 0,
                            stop=c == n_kc - 1,
                        )
                    # denominator: pair-tiles 0-6 accumulate on DVE; the
                    # accs and the last tile fold on PE at block end.
                    if p == n_pr - 1:
                        last_p0 = p0
                    else:
                        ai = 0 if p < 4 else 1
                        if accs[ai] is None:
                            accs[ai] = p0  # first tile is the seed (no op)
                            seeds[ai] = p0
                        elif seeds[ai] is not None:
                            t2 = apool.tile([KP, 2 * QB], bf16, tag="acc")
                            nc.vector.tensor_tensor(
                                t2[:], seeds[ai][:], p0[:], mybir.AluOpType.add
                            )
                            accs[ai] = t2
                            seeds[ai] = None
                        else:
                            nc.vector.tensor_tensor(
                                accs[ai][:], accs[ai][:], p0[:],
                                mybir.AluOpType.add,
                            )

                # fold the two accumulators, then the last tile (stop)
                n_fs = 3
                for fi, src in enumerate((accs[0], accs[1], last_p0)):
                    for half in range(2):
                        nc.tensor.matmul(
                            oden[:],
                            ones,
                            src[:, half * QB : (half + 1) * QB],
                            start=fi == 0 and half == 0,
                            stop=fi == n_fs - 1 and half == 1,
                        )

                # out = oacc / (0.5 * sum exp). Deferred into the next block's
                # DVE stream (after its first mask-mult) so the recip's wait
                # on the PE fold chain never blocks the DVE queue head;
                # oacc/oden are double-buffered so no PSUM WAR either way.
                def make_out(h=h, qb=qb, oacc=oacc, oden=oden):
                    def emit():
                        q0 = qb * QB
                        rb = opool.tile([d, QB], f32, tag="rb")
                        nc.vector.reciprocal_approx_fast(rb[:], oden[:])
                        out_sb = opool.tile([d, QB], f32, tag="out")
                        nc.vector.tensor_tensor(
                            out_sb[:], oacc[:], rb[:], mybir.AluOpType.mult
                        )

                        def emit_dma():
                            nc.sync.dma_start(ot_d[h, :, q0 : q0 + QB], out_sb[:])

                        return emit_dma

                    return emit

                pending[0] = make_out()

            if pending[0] is not None:
                pending[1] = pending[0]()
            if pending[1] is not None:
                pending[1]()

    nc.compile()
    return nc


_CACHE: dict = {}


def _get_program(scale: float):
    key = float(scale)
    if key not in _CACHE:
        _CACHE[key] = build_program(scale=key)
    return _CACHE[key]


def make_in_maps(query, key, value, dropout_mask, **_ignored):
    """Shard + relayout the full inputs into the 8 per-core input maps."""
    import ml_dtypes

    query = np.asarray(query, dtype=np.float32)
    key = np.asarray(key, dtype=np.float32)
    value = np.asarray(value, dtype=np.float32)
    dropout_mask = np.asarray(dropout_mask, dtype=np.float32)
    in_maps = []
    for cid in range(N_CORES):
        sl = slice(cid * HPC, (cid + 1) * HPC)
        qt = np.ascontiguousarray(query[sl].transpose(0, 2, 1))  # [h, d, S]
        qt2 = np.concatenate([qt, qt], axis=1)  # [h, 128, S] duplicated
        kt = key[sl].transpose(0, 2, 1)  # [h, d, S]
        # even chunks -> rows 0-63, odd chunks -> rows 64-127
        kt2 = np.ascontiguousarray(
            kt.reshape(HPC, D, S // KP // 2, 2, KP)
            .transpose(0, 3, 1, 2, 4)
            .reshape(HPC, 2 * D, S // 2)
        )
        vp = np.ascontiguousarray(
            value[sl].reshape(HPC, S // KP, KP, D).transpose(0, 2, 1, 3)
        ).reshape(HPC, KP, (S // KP) * D).astype(ml_dtypes.bfloat16)
        mt = (dropout_mask[sl].transpose(0, 2, 1) >= DROP_P).astype(
            ml_dtypes.bfloat16
        )  # [h, k, q] keep-mask
        in_maps.append({"qt": qt2, "kt": kt2, "vp": vp, "mt": mt})
    return in_maps


def run(query, key, value, scale_factor, dropout_mask, trace=False, **trace_kwargs):
    scale = float(np.asarray(scale_factor).reshape(()))
    nc = _get_program(scale)
    in_maps = make_in_maps(query, key, value, dropout_mask)
    res = run_bass_kernel_spmd(
        nc, in_maps, core_ids=list(range(N_CORES)), trace=trace, **trace_kwargs
    )
    outs = [res.results[c]["ot"].transpose(0, 2, 1) for c in range(N_CORES)]
    full = np.ascontiguousarray(np.concatenate(outs, axis=0), dtype=np.float32)
    return full, res


def kernel(query, key, value, scale_factor, dropout_mask):
    out, _ = run(query, key, value, scale_factor, dropout_mask, trace=False)
    return out
